# revision 18
# baseline (speedup 1.0000x reference)
"""Trainium2 Bass kernel for CoocOpModel.

out[b,s,z] = sum_{i,j} func[b,s,i] * cooc[i,j,z] * arg[b,s,j]
  with func = func_and_arg[..., :128], arg = func_and_arg[..., 128:]

Shapes (hardcoded): func_and_arg [4,1024,256] f32, cooccurrences [128,128,128] f32,
out [4,1024,128] f32.  D = 128, tokens T = 4096.

Strategy: data-parallel over tokens across 8 cores (512 tokens/core);
cooccurrence tensor replicated per core (fp16).

Per-core math, with t = local token index (512), i/j/z in [0,128):
  out_T[z, t] = sum_i  C_i^T @ G_i        (accumulated in one PSUM bank)
  C_i[j, z]   = cooc[i, j, z]             (stationary operand, fp16)
  G_i[j, t]   = arg_T[j, t] * func_T[i, t]  (moving operand, fp16)

i's are processed in groups of GRP=8. Each group needs
f_exp_g[j, (k,t)] = func_T[8g+k, t] replicated across the 128 j-partitions
(1 MB fp16, 16 MB total). The DMA bus saturates at ~360-420 GB/s/core with
a single queue (more queues don't scale), so only ~11 groups can come from
DMA broadcast. The rest (ACT_GROUPS) are built on chip:
  PE matmul (ones[1,128] stationary x func row [1,512] moving) broadcasts a
  row into a PSUM chunk; the otherwise-idle Act engine copies PSUM->SBUF
  fp16. This costs ~1.8us PE + ~4us Act per group and zero DMA/DVE time.
GpSimd is left idle on purpose: bulk pool ops slow concurrent DVE ~3x.
The G multiplies all run on DVE (fp16 2x mode, ~2.3-2.6 us/group).
"""

import sys

sys.path.insert(0, "/opt/trn_rl_repo")

import numpy as np
from contextlib import ExitStack

import concourse.bass as bass
import concourse.tile as tile
from concourse import bacc, mybir
from concourse.bass_utils import run_bass_kernel_spmd

F16 = mybir.dt.float16
F32 = mybir.dt.float32
NP_F16 = np.float16

N_CORES = 8
D = 128
T_TOTAL = 4096
T_CORE = T_TOTAL // N_CORES  # 512
GRP = 8
N_GRP = D // GRP  # 16

ACT_GROUPS = (3, 6, 9, 12, 15)  # f_exp via PE-broadcast + Act copy
CHUNK_I = 2  # i-rows per PSUM chunk ([128, 2*512] f32 = 2 banks)

_NC_CACHE = None


def _build():
    nc = bacc.Bacc("TRN2", target_bir_lowering=False, debug=False, num_devices=N_CORES)

    f_t = nc.dram_tensor("f_t", [D, T_CORE], F16, kind="ExternalInput").ap()
    a_t = nc.dram_tensor("a_t", [D, T_CORE], F16, kind="ExternalInput").ap()
    # c2[j, i*128 + z] = cooc[i, j, z]
    c2 = nc.dram_tensor("c2", [D, D * D], F16, kind="ExternalInput").ap()
    out_t = nc.dram_tensor("out_t", [D, T_CORE], F32, kind="ExternalOutput").ap()

    with tile.TileContext(nc) as tc:
        with ExitStack() as ctx:
            const_pool = ctx.enter_context(tc.tile_pool(name="const", bufs=1))
            fexp_pool = ctx.enter_context(tc.tile_pool(name="fexp", bufs=11))
            fact_pool = ctx.enter_context(tc.tile_pool(name="fact", bufs=3))
            g_pool = ctx.enter_context(tc.tile_pool(name="g", bufs=3))
            out_pool = ctx.enter_context(tc.tile_pool(name="out", bufs=1))
            psum_pool = ctx.enter_context(
                tc.tile_pool(name="psum", bufs=1, space="PSUM")
            )
            chunk_pool = ctx.enter_context(
                tc.tile_pool(name="chunk", bufs=3, space="PSUM")
            )

            # ones rows for PE broadcast (stationary [1, 128] at base
            # partitions 0/32/64 to match the staged moving operands)
            ones_sb = const_pool.tile([65, D], F16, tag="ones")
            nc.gpsimd.memset(ones_sb[:], 1.0)

            # act-group func rows staged onto base partitions 0/32/64
            # (matmul operands must sit at base partition 0/32/64; two
            # groups share a partition to halve the SBUF column).
            # staging DMAs go on the gpsimd queue: tiny transfers with big
            # fixed overheads would otherwise stall a main queue's head.
            n_act = len(ACT_GROUPS)
            GT = GRP * T_CORE
            stage = const_pool.tile([65, 2 * GT], F16, tag="stage")
            act_idx = {g: k for k, g in enumerate(ACT_GROUPS)}
            stage_loc = {}  # g -> (base_partition, offset)
            for g in ACT_GROUPS:
                k = act_idx[g]
                bp, off = (k // 2) * 32, (k % 2) * GT
                stage_loc[g] = (bp, off)
                src = bass.AP(
                    f_t.tensor, g * GT, [[T_CORE, GRP], [1, T_CORE]]
                )
                nc.gpsimd.dma_start(stage[bp : bp + 1, off : off + GT], src)

            # arg_T in SBUF; the TT re-reads it per k via a free-step-0 AP.
            a_sb = const_pool.tile([D, T_CORE], F16, tag="a")
            nc.sync.dma_start(a_sb[:], a_t[:, :])
            a_ap = a_sb[:]

            # all cooc tiles are statically allocated; issue DMAs early &
            # interleaved.  c_sb[g][j, (k, z)] = cooc[8g+k, j, z]
            c_tiles = [
                const_pool.tile([D, GRP * D], F16, tag=f"c{g}", name=f"c_sb{g}")
                for g in range(N_GRP)
            ]

            # --- act-group chunk machinery -------------------------------
            # Each act group g needs f_exp[j,(k,t)] = f_sb[8g+k, t].
            # 4 chunks of CHUNK_I=2 rows: PE bcast-MM pair -> PSUM chunk,
            # Act copies chunk -> fp16 SBUF tile slice.
            fact_tiles = {}

            act_units = []  # (g, chunk_idx)
            for g in ACT_GROUPS:
                for c in range(GRP // CHUNK_I):
                    act_units.append((g, c))
            unit_pos = 0

            def emit_act_units(n):
                nonlocal unit_pos
                for _ in range(n):
                    if unit_pos >= len(act_units):
                        return
                    g, c = act_units[unit_pos]
                    unit_pos += 1
                    if c == 0:
                        fact_tiles[g] = fact_pool.tile(
                            [D, GRP * T_CORE], F16, tag="fact", name=f"fact{g}"
                        )
                    ch = chunk_pool.tile(
                        [D, CHUNK_I * T_CORE], F32, tag="chunk", name=f"chunk{g}_{c}"
                    )
                    bp, goff = stage_loc[g]
                    for k in range(CHUNK_I):
                        off = goff + (c * CHUNK_I + k) * T_CORE
                        nc.tensor.matmul(
                            ch[:, k * T_CORE : (k + 1) * T_CORE],
                            ones_sb[bp : bp + 1, :],
                            stage[bp : bp + 1, off : off + T_CORE],
                            start=True,
                            stop=True,
                        )
                    nc.scalar.copy(
                        fact_tiles[g][
                            :,
                            c * CHUNK_I * T_CORE : (c + 1) * CHUNK_I * T_CORE,
                        ],
                        ch[:],
                    )

            # greedy byte-balanced assignment across the two HWDGE queues
            q_bytes = [0, 0]
            dma_q = [nc.sync, nc.scalar]

            def issue(dst, src, nbytes, q=None):
                if q is None:
                    q = 0 if q_bytes[0] <= q_bytes[1] else 1
                dma_q[q].dma_start(dst, src)
                q_bytes[q] += nbytes

            FB = D * GRP * T_CORE * 2  # f_exp bytes per group (1 MB)
            CB = D * GRP * D * 2  # cooc bytes per group (256 KB)

            # ---- issue ALL DMA dispatches upfront -----------------------
            # The Act engine both dispatches DMAs and runs the PSUM->SBUF
            # copies; dispatches emitted after a copy would sit behind it
            # (head-of-line in the engine program) and starve the queue.
            # With fexp bufs=11 (every DMA group its own tile) no dispatch
            # has a WAR wait, so both engine streams are pure dispatch runs
            # and the HW queues drain asynchronously at full bus rate.
            fexp_tiles = {}
            cooc_issued = 0

            def issue_cooc_n(n):
                nonlocal cooc_issued
                for _ in range(n):
                    if cooc_issued >= N_GRP:
                        return
                    g = cooc_issued
                    cooc_issued += 1
                    issue(c_tiles[g][:], c2[:, g * GRP * D : (g + 1) * GRP * D], CB)

            for g in range(N_GRP):
                if g in ACT_GROUPS:
                    issue_cooc_n(1)
                    continue
                f_exp = fexp_pool.tile(
                    [D, GRP * T_CORE], F16, tag="fexp", name=f"fexp{g}"
                )
                fexp_tiles[g] = f_exp
                if g == 0:
                    half = GRP // 2
                    f_src_a = bass.AP(
                        f_t.tensor, 0, [[0, D], [T_CORE, half], [1, T_CORE]]
                    )
                    f_src_b = bass.AP(
                        f_t.tensor,
                        half * T_CORE,
                        [[0, D], [T_CORE, half], [1, T_CORE]],
                    )
                    issue(f_exp[:, : half * T_CORE], f_src_a, FB // 2, q=1)
                    issue(f_exp[:, half * T_CORE :], f_src_b, FB // 2, q=0)
                else:
                    f_src = bass.AP(
                        f_t.tensor,
                        g * GRP * T_CORE,
                        [[0, D], [T_CORE, GRP], [1, T_CORE]],
                    )
                    issue(f_exp[:], f_src, FB)
                issue_cooc_n(2)

            ps = psum_pool.tile([D, T_CORE], F32)
            for g in range(N_GRP):
                i0 = g * GRP
                sz = GRP

                # front-load PE broadcast + Act copies (2 units per group)
                emit_act_units(2)

                src_tile = fact_tiles[g] if g in ACT_GROUPS else fexp_tiles[g]

                a_view = bass.AP(
                    a_ap.tensor, a_ap.offset, [a_ap.ap[0], [0, sz], [1, T_CORE]]
                )
                gt = g_pool.tile([D, sz * T_CORE], F16, tag="g")
                if g == 0 or g == N_GRP - 1:
                    h = sz // 2
                    a_half = bass.AP(
                        a_ap.tensor, a_ap.offset, [a_ap.ap[0], [0, h], [1, T_CORE]]
                    )
                    nc.vector.tensor_mul(
                        gt[:, : h * T_CORE], a_half, src_tile[:, : h * T_CORE]
                    )
                    nc.vector.tensor_mul(
                        gt[:, h * T_CORE :], a_half, src_tile[:, h * T_CORE :]
                    )
                else:
                    nc.vector.tensor_mul(gt[:], a_view, src_tile[:])

                for k in range(sz):
                    i = i0 + k
                    nc.tensor.matmul(
                        ps[:],
                        c_tiles[g][:, k * D : (k + 1) * D],
                        gt[:, k * T_CORE : (k + 1) * T_CORE],
                        start=(i == 0),
                        stop=(i == D - 1),
                    )

            o_sb = out_pool.tile([D, T_CORE], F32, tag="o")
            nc.scalar.copy(o_sb[:], ps[:])
            nc.sync.dma_start(out_t[:, :], o_sb[:])

    nc.compile()
    return nc


def _get_nc():
    global _NC_CACHE
    if _NC_CACHE is None:
        _NC_CACHE = _build()
    return _NC_CACHE


def _prep_in_maps(func_and_arg, cooccurrences):
    fa = np.asarray(func_and_arg, dtype=np.float32).reshape(T_TOTAL, 2 * D)
    c2 = (
        np.ascontiguousarray(
            np.asarray(cooccurrences, dtype=np.float32).transpose(1, 0, 2)
        )
        .reshape(D, D * D)
        .astype(NP_F16)
    )
    in_maps = []
    for c in range(N_CORES):
        s = fa[c * T_CORE : (c + 1) * T_CORE]  # [512, 256]
        f_tc = np.ascontiguousarray(s[:, :D].T).astype(NP_F16)  # [128 i, 512 t]
        a_tc = np.ascontiguousarray(s[:, D:].T).astype(NP_F16)  # [128 j, 512 t]
        in_maps.append({"f_t": f_tc, "a_t": a_tc, "c2": c2})
    return in_maps


def kernel(func_and_arg: np.ndarray, cooccurrences: np.ndarray) -> np.ndarray:
    assert func_and_arg.shape == (4, 1024, 2 * D)
    assert cooccurrences.shape == (D, D, D)

    in_maps = _prep_in_maps(func_and_arg, cooccurrences)
    nc = _get_nc()
    res = run_bass_kernel_spmd(nc, in_maps, core_ids=list(range(N_CORES)))

    # out_t per core: [z=128, t=512] -> [t, z]; concat over cores -> [4096, 128]
    outs = [res.results[c]["out_t"].T for c in range(N_CORES)]
    out = np.concatenate(outs, axis=0).reshape(4, 1024, D).astype(np.float32)
    return out


# revision 22
# speedup vs baseline: 1.0250x; 1.0250x over previous
"""Trainium2 Bass kernel for CoocOpModel.

out[b,s,z] = sum_{i,j} func[b,s,i] * cooc[i,j,z] * arg[b,s,j]
  with func = func_and_arg[..., :128], arg = func_and_arg[..., 128:]

Shapes (hardcoded): func_and_arg [4,1024,256] f32, cooccurrences [128,128,128] f32,
out [4,1024,128] f32.  D = 128, tokens T = 4096.

Strategy: data-parallel over tokens across 8 cores (512 tokens/core);
cooccurrence tensor replicated per core (fp16).

Per-core math, with t = local token index (512), i/j/z in [0,128):
  out_T[z, t] = sum_i  C_i^T @ G_i        (accumulated in one PSUM bank)
  C_i[j, z]   = cooc[i, j, z]             (stationary operand, fp16)
  G_i[j, t]   = arg_T[j, t] * func_T[i, t]  (moving operand, fp16)

i's are processed in groups of GRP=8. Each group needs
f_exp_g[j, (k,t)] = func_T[8g+k, t] replicated across the 128 j-partitions
(1 MB fp16, 16 MB total). The DMA bus saturates at ~360-420 GB/s/core with
a single queue (more queues don't scale), so only ~11 groups can come from
DMA broadcast. The rest (ACT_GROUPS) are built on chip:
  PE matmul (ones[1,128] stationary x func row [1,512] moving) broadcasts a
  row into a PSUM chunk; the otherwise-idle Act engine copies PSUM->SBUF
  fp16. This costs ~1.8us PE + ~4us Act per group and zero DMA/DVE time.
GpSimd is left idle on purpose: bulk pool ops slow concurrent DVE ~3x.
The G multiplies all run on DVE (fp16 2x mode, ~2.3-2.6 us/group).
"""

import sys

sys.path.insert(0, "/opt/trn_rl_repo")

import numpy as np
from contextlib import ExitStack

import concourse.bass as bass
import concourse.tile as tile
from concourse import bacc, mybir
from concourse.bass_utils import run_bass_kernel_spmd

F16 = mybir.dt.float16
F32 = mybir.dt.float32
NP_F16 = np.float16

N_CORES = 8
D = 128
T_TOTAL = 4096
T_CORE = T_TOTAL // N_CORES  # 512
GRP = 8
N_GRP = D // GRP  # 16

ACT_GROUPS = (5, 8, 10, 13, 15)  # f_exp via PE-broadcast + Act copy
CHUNK_I = 2  # i-rows per PSUM chunk ([128, 2*512] f32 = 2 banks)

_NC_CACHE = None


def _build():
    nc = bacc.Bacc("TRN2", target_bir_lowering=False, debug=False, num_devices=N_CORES)

    f_t = nc.dram_tensor("f_t", [D, T_CORE], F16, kind="ExternalInput").ap()
    a_t = nc.dram_tensor("a_t", [D, T_CORE], F16, kind="ExternalInput").ap()
    # c2[j, i*128 + z] = cooc[i, j, z]
    c2 = nc.dram_tensor("c2", [D, D * D], F16, kind="ExternalInput").ap()
    out_t = nc.dram_tensor("out_t", [D, T_CORE], F32, kind="ExternalOutput").ap()

    with tile.TileContext(nc) as tc:
        with ExitStack() as ctx:
            const_pool = ctx.enter_context(tc.tile_pool(name="const", bufs=1))
            fexp_pool = ctx.enter_context(tc.tile_pool(name="fexp", bufs=11))
            fact_pool = ctx.enter_context(tc.tile_pool(name="fact", bufs=3))
            g_pool = ctx.enter_context(tc.tile_pool(name="g", bufs=3))
            out_pool = ctx.enter_context(tc.tile_pool(name="out", bufs=1))
            psum_pool = ctx.enter_context(
                tc.tile_pool(name="psum", bufs=1, space="PSUM")
            )
            chunk_pool = ctx.enter_context(
                tc.tile_pool(name="chunk", bufs=3, space="PSUM")
            )

            # ones rows for PE broadcast (stationary [1, 128] at base
            # partitions 0/32/64 to match the staged moving operands)
            ones_sb = const_pool.tile([65, D], F16, tag="ones")
            nc.gpsimd.memset(ones_sb[:], 1.0)

            # act-group func rows staged onto base partitions 0/32/64
            # (matmul operands must sit at base partition 0/32/64; two
            # groups share a partition to halve the SBUF column).
            # The groups are chosen 5 apart so that all staging fits in TWO
            # strided dma_starts (tiny per-dispatch transfers are slow, so
            # minimize their count) at the head of the sync queue.
            GT = GRP * T_CORE
            stage = const_pool.tile([65, 2 * GT], F16, tag="stage")
            stage_loc = {5: (0, 0), 10: (32, 0), 15: (64, 0), 8: (0, GT), 13: (32, GT)}
            assert set(ACT_GROUPS) == set(stage_loc)
            p_pitch = stage[:].ap[0][0]
            src1 = bass.AP(
                f_t.tensor, 5 * GT, [[5 * GT, 3], [T_CORE, GRP], [1, T_CORE]]
            )
            dst1 = bass.AP(stage[:].tensor, stage[:].offset, [[32 * p_pitch, 3], [1, GT]])
            nc.sync.dma_start(dst1, src1)
            src2 = bass.AP(
                f_t.tensor, 8 * GT, [[5 * GT, 2], [T_CORE, GRP], [1, T_CORE]]
            )
            dst2 = bass.AP(
                stage[:].tensor, stage[:].offset + GT, [[32 * p_pitch, 2], [1, GT]]
            )
            nc.sync.dma_start(dst2, src2)

            # arg_T in SBUF; the TT re-reads it per k via a free-step-0 AP.
            a_sb = const_pool.tile([D, T_CORE], F16, tag="a")
            nc.sync.dma_start(a_sb[:], a_t[:, :])
            a_ap = a_sb[:]

            # all cooc tiles are statically allocated; issue DMAs early &
            # interleaved.  c_sb[g][j, (k, z)] = cooc[8g+k, j, z]
            c_tiles = [
                const_pool.tile([D, GRP * D], F16, tag=f"c{g}", name=f"c_sb{g}")
                for g in range(N_GRP)
            ]

            # --- act-group chunk machinery -------------------------------
            # Each act group g needs f_exp[j,(k,t)] = f_sb[8g+k, t].
            # 4 chunks of CHUNK_I=2 rows: PE bcast-MM pair -> PSUM chunk,
            # Act copies chunk -> fp16 SBUF tile slice.
            fact_tiles = {}

            act_units = []  # (g, chunk_idx)
            for g in ACT_GROUPS:
                for c in range(GRP // CHUNK_I):
                    act_units.append((g, c))
            unit_pos = 0

            def emit_act_units(n):
                nonlocal unit_pos
                for _ in range(n):
                    if unit_pos >= len(act_units):
                        return
                    g, c = act_units[unit_pos]
                    unit_pos += 1
                    if c == 0:
                        fact_tiles[g] = fact_pool.tile(
                            [D, GRP * T_CORE], F16, tag="fact", name=f"fact{g}"
                        )
                    ch = chunk_pool.tile(
                        [D, CHUNK_I * T_CORE], F32, tag="chunk", name=f"chunk{g}_{c}"
                    )
                    bp, goff = stage_loc[g]
                    for k in range(CHUNK_I):
                        off = goff + (c * CHUNK_I + k) * T_CORE
                        nc.tensor.matmul(
                            ch[:, k * T_CORE : (k + 1) * T_CORE],
                            ones_sb[bp : bp + 1, :],
                            stage[bp : bp + 1, off : off + T_CORE],
                            start=True,
                            stop=True,
                        )
                    nc.scalar.copy(
                        fact_tiles[g][
                            :,
                            c * CHUNK_I * T_CORE : (c + 1) * CHUNK_I * T_CORE,
                        ],
                        ch[:],
                    )

            # greedy byte-balanced assignment across the two HWDGE queues
            q_bytes = [0, 0]
            dma_q = [nc.sync, nc.scalar]

            def issue(dst, src, nbytes, q=None):
                if q is None:
                    q = 0 if q_bytes[0] <= q_bytes[1] else 1
                dma_q[q].dma_start(dst, src)
                q_bytes[q] += nbytes

            FB = D * GRP * T_CORE * 2  # f_exp bytes per group (1 MB)
            CB = D * GRP * D * 2  # cooc bytes per group (256 KB)

            # ---- issue ALL DMA dispatches upfront -----------------------
            # The Act engine both dispatches DMAs and runs the PSUM->SBUF
            # copies; dispatches emitted after a copy would sit behind it
            # (head-of-line in the engine program) and starve the queue.
            # With fexp bufs=11 (every DMA group its own tile) no dispatch
            # has a WAR wait, so both engine streams are pure dispatch runs
            # and the HW queues drain asynchronously at full bus rate.
            fexp_tiles = {}
            cooc_issued = 0

            def issue_cooc_n(n):
                nonlocal cooc_issued
                for _ in range(n):
                    if cooc_issued >= N_GRP:
                        return
                    g = cooc_issued
                    cooc_issued += 1
                    issue(c_tiles[g][:], c2[:, g * GRP * D : (g + 1) * GRP * D], CB)

            for g in range(N_GRP):
                if g in ACT_GROUPS:
                    issue_cooc_n(1)
                    continue
                # (1 cooc after each f_exp keeps cooc slightly ahead of its
                # consumer without delaying the f_exp stream)
                f_exp = fexp_pool.tile(
                    [D, GRP * T_CORE], F16, tag="fexp", name=f"fexp{g}"
                )
                fexp_tiles[g] = f_exp
                if g == 0:
                    half = GRP // 2
                    f_src_a = bass.AP(
                        f_t.tensor, 0, [[0, D], [T_CORE, half], [1, T_CORE]]
                    )
                    f_src_b = bass.AP(
                        f_t.tensor,
                        half * T_CORE,
                        [[0, D], [T_CORE, half], [1, T_CORE]],
                    )
                    issue(f_exp[:, : half * T_CORE], f_src_a, FB // 2, q=1)
                    issue(f_exp[:, half * T_CORE :], f_src_b, FB // 2, q=0)
                else:
                    f_src = bass.AP(
                        f_t.tensor,
                        g * GRP * T_CORE,
                        [[0, D], [T_CORE, GRP], [1, T_CORE]],
                    )
                    issue(f_exp[:], f_src, FB)
                issue_cooc_n(1)

            ps = psum_pool.tile([D, T_CORE], F32)
            for g in range(N_GRP):
                i0 = g * GRP
                sz = GRP

                # front-load PE broadcast + Act copies (2 units per group)
                emit_act_units(2)

                src_tile = fact_tiles[g] if g in ACT_GROUPS else fexp_tiles[g]

                a_view = bass.AP(
                    a_ap.tensor, a_ap.offset, [a_ap.ap[0], [0, sz], [1, T_CORE]]
                )
                gt = g_pool.tile([D, sz * T_CORE], F16, tag="g")
                if g == 0 or g == N_GRP - 1:
                    h = sz // 2
                    a_half = bass.AP(
                        a_ap.tensor, a_ap.offset, [a_ap.ap[0], [0, h], [1, T_CORE]]
                    )
                    nc.vector.tensor_mul(
                        gt[:, : h * T_CORE], a_half, src_tile[:, : h * T_CORE]
                    )
                    nc.vector.tensor_mul(
                        gt[:, h * T_CORE :], a_half, src_tile[:, h * T_CORE :]
                    )
                else:
                    nc.vector.tensor_mul(gt[:], a_view, src_tile[:])

                for k in range(sz):
                    i = i0 + k
                    nc.tensor.matmul(
                        ps[:],
                        c_tiles[g][:, k * D : (k + 1) * D],
                        gt[:, k * T_CORE : (k + 1) * T_CORE],
                        start=(i == 0),
                        stop=(i == D - 1),
                    )

            o_sb = out_pool.tile([D, T_CORE], F32, tag="o")
            nc.scalar.copy(o_sb[:], ps[:])
            nc.sync.dma_start(out_t[:, :], o_sb[:])

    nc.compile()
    return nc


def _get_nc():
    global _NC_CACHE
    if _NC_CACHE is None:
        _NC_CACHE = _build()
    return _NC_CACHE


def _prep_in_maps(func_and_arg, cooccurrences):
    fa = np.asarray(func_and_arg, dtype=np.float32).reshape(T_TOTAL, 2 * D)
    c2 = (
        np.ascontiguousarray(
            np.asarray(cooccurrences, dtype=np.float32).transpose(1, 0, 2)
        )
        .reshape(D, D * D)
        .astype(NP_F16)
    )
    in_maps = []
    for c in range(N_CORES):
        s = fa[c * T_CORE : (c + 1) * T_CORE]  # [512, 256]
        f_tc = np.ascontiguousarray(s[:, :D].T).astype(NP_F16)  # [128 i, 512 t]
        a_tc = np.ascontiguousarray(s[:, D:].T).astype(NP_F16)  # [128 j, 512 t]
        in_maps.append({"f_t": f_tc, "a_t": a_tc, "c2": c2})
    return in_maps


def kernel(func_and_arg: np.ndarray, cooccurrences: np.ndarray) -> np.ndarray:
    assert func_and_arg.shape == (4, 1024, 2 * D)
    assert cooccurrences.shape == (D, D, D)

    in_maps = _prep_in_maps(func_and_arg, cooccurrences)
    nc = _get_nc()
    res = run_bass_kernel_spmd(nc, in_maps, core_ids=list(range(N_CORES)))

    # out_t per core: [z=128, t=512] -> [t, z]; concat over cores -> [4096, 128]
    outs = [res.results[c]["out_t"].T for c in range(N_CORES)]
    out = np.concatenate(outs, axis=0).reshape(4, 1024, D).astype(np.float32)
    return out
